# revision 9
# baseline (speedup 1.0000x reference)
"""Block-diagonal linear (grouped GEMM) on 8 TRN2 NeuronCores.

out[b, g*512+n] = sum_k x[b, g*512+k] * blocks[g, k, n]

Sharding: group-parallel — core g computes block g's GEMM. The host hands
each core xT = x[:, g*512:(g+1)*512].T ([512, 8192], feature-major) cast to
bf16 and receives outT ([512, 8192], bf16); transposes/casts happen on the
host so the device needs no PE transposes and every DMA stream reads/writes
long contiguous runs per partition.

bf16 everywhere: matmul runs at the full 1 col/cycle PE rate (same as f32r)
while halving HBM traffic vs fp32 (16.8 MB/core total, under the ~47 us DMA
floor), so the kernel is PE-bound at ~55 us. End-to-end rel err ~4e-3
(fp32 PSUM accumulation), well inside the 2e-2 gate.

Per-core kernel: out.T = W.T @ x.T as 64 PSUM accumulation groups:
psum[n-tile 128, m 512] += W[k-tile, n-tile].T @ xT[k-tile, m-chunk].
"""
import numpy as np
import ml_dtypes

import concourse.bacc as bacc
import concourse.tile as tile
from concourse import mybir
from concourse.bass_utils import run_bass_kernel_spmd

TOKENS = 8192
G = 8
M = 512  # per-block in-features
N = 512  # per-block out-features
P = 128
KT = M // P  # 4 contraction tiles
NT = N // P  # 4 output feature tiles
SUB = 512    # tokens per PSUM group (moving-dim max)
F32 = mybir.dt.float32
BF16 = mybir.dt.bfloat16
NP_BF16 = ml_dtypes.bfloat16

# token-chunk schedule: chunk 0 arrives via SWDGE (fast descriptor gen),
# later chunks stream on the two HWDGE rings; small tail for a quick flush
CHUNKS = [512, 1024, 1024, 2048, 2048, 1024, 384, 128]
assert sum(CHUNKS) == TOKENS
CMAX = max(CHUNKS)

_CACHE: dict = {}


def _body(tc, nc, xT, w, outT):
    with (
        tc.tile_pool(name="wp", bufs=1) as wp,
        tc.tile_pool(name="xin", bufs=20) as xin,
        tc.tile_pool(name="outp", bufs=2) as outp,
        tc.tile_pool(name="pso", bufs=8, space="PSUM") as pso,
    ):
        # weights [512, 512] bf16 -> [128, kt, 512], one batched SWDGE DMA:
        # software descriptor gen is ~60x cheaper per descriptor than the
        # HWDGE rings, so W lands ~5us earlier than it would on sync/scalar
        w_t = wp.tile([P, KT, N], BF16, tag="wf")
        nc.gpsimd.dma_start(w_t[:], w.rearrange("(j p) n -> p j n", p=P))

        # chunk 0 likewise rides SWDGE as a single batched DMA
        x_pj = xT.rearrange("(j p) m -> p j m", p=P)
        x0_t = wp.tile([P, KT, CHUNKS[0]], BF16, tag="x0")
        nc.gpsimd.dma_start(x0_t[:], x_pj[:, :, 0:CHUNKS[0]])

        m0 = 0
        for ci, c in enumerate(CHUNKS):
            if ci == 0:
                xs = [x0_t[:, j, :] for j in range(KT)]
            else:
                # later chunks stream on the two HWDGE rings (sync=SP,
                # scalar=ACT), two k-tiles per ring
                xs = []
                for j in range(KT):
                    x_t = xin.tile([P, CMAX], BF16, tag="x")
                    eng = nc.sync if j % 2 == 0 else nc.scalar
                    eng.dma_start(x_t[:, :c], xT[j * P:(j + 1) * P, m0:m0 + c])
                    xs.append(x_t[:, :])

            ots = [outp.tile([P, CMAX], BF16, tag=f"o{nt}", name=f"ot{nt}") for nt in range(NT)]
            for s0 in range(0, c, SUB):
                sw = min(SUB, c - s0)
                for nt in range(NT):
                    ps_o = pso.tile([P, SUB], F32, tag="pso")
                    for j in range(KT):
                        nc.tensor.matmul(
                            ps_o[:, :sw],
                            w_t[:, j, nt * P:(nt + 1) * P],
                            xs[j][:, s0:s0 + sw],
                            start=(j == 0),
                            stop=(j == KT - 1),
                        )
                    # split the PSUM drain into two half-tiles on DVE + ACT so
                    # the bank recycles ~2x sooner (frees the PE 2 groups on)
                    h = sw // 2
                    nc.vector.tensor_copy(ots[nt][:, s0:s0 + h], ps_o[:, :h])
                    nc.scalar.copy(ots[nt][:, s0 + h:s0 + sw], ps_o[:, h:sw])
            # flush the chunk on the SWDGE ring; the last two chunks ride the
            # HWDGE rings (input traffic is done by then) so the SWDGE queue
            # drains while the tail still computes
            for nt in range(NT):
                if ci >= len(CHUNKS) - 2:
                    eng = nc.sync if nt % 2 == 0 else nc.scalar
                else:
                    eng = nc.gpsimd
                eng.dma_start(outT[nt * P:(nt + 1) * P, m0:m0 + c], ots[nt][:, :c])
            m0 += c


def _build():
    nc = bacc.Bacc("TRN2", target_bir_lowering=False, debug=False, num_devices=G)
    xT = nc.dram_tensor("xT", [M, TOKENS], BF16, kind="ExternalInput").ap()
    w = nc.dram_tensor("w", [M, N], BF16, kind="ExternalInput").ap()
    outT = nc.dram_tensor("outT", [N, TOKENS], BF16, kind="ExternalOutput").ap()
    with tile.TileContext(nc) as tc:
        _body(tc, nc, xT, w, outT)
    nc.compile()
    return nc


def _run(in_maps, **kwargs):
    if "nc" not in _CACHE:
        _CACHE["nc"] = _build()
    return run_bass_kernel_spmd(_CACHE["nc"], in_maps, list(range(G)), **kwargs)


def _in_maps(x, blocks):
    return [
        {
            "xT": np.ascontiguousarray(x[:, g * M:(g + 1) * M].T).astype(NP_BF16),
            "w": np.ascontiguousarray(blocks[g]).astype(NP_BF16),
        }
        for g in range(G)
    ]


def kernel(x, blocks):
    x = np.asarray(x, dtype=np.float32)
    blocks = np.asarray(blocks, dtype=np.float32)
    res = _run(_in_maps(x, blocks))
    return np.concatenate(
        [res.results[g]["outT"].T for g in range(G)], axis=1
    ).astype(np.float32)


# revision 11
# speedup vs baseline: 1.0129x; 1.0129x over previous
"""Block-diagonal linear (grouped GEMM) on 8 TRN2 NeuronCores.

out[b, g*512+n] = sum_k x[b, g*512+k] * blocks[g, k, n]

Sharding: group-parallel — core g computes block g's GEMM. The host hands
each core xT = x[:, g*512:(g+1)*512].T ([512, 8192], feature-major) cast to
bf16 and receives outT ([512, 8192], bf16); transposes/casts happen on the
host so the device needs no PE transposes and every DMA stream reads/writes
long contiguous runs per partition.

bf16 everywhere: matmul runs at the full 1 col/cycle PE rate (same as f32r)
while halving HBM traffic vs fp32 (16.8 MB/core total, under the ~47 us DMA
floor), so the kernel is PE-bound at ~55 us. End-to-end rel err ~4e-3
(fp32 PSUM accumulation), well inside the 2e-2 gate.

Per-core kernel: out.T = W.T @ x.T as 64 PSUM accumulation groups:
psum[n-tile 128, m 512] += W[k-tile, n-tile].T @ xT[k-tile, m-chunk].
"""
import numpy as np
import ml_dtypes

import concourse.bacc as bacc
import concourse.tile as tile
from concourse import mybir
from concourse.bass_utils import run_bass_kernel_spmd

TOKENS = 8192
G = 8
M = 512  # per-block in-features
N = 512  # per-block out-features
P = 128
KT = M // P  # 4 contraction tiles
NT = N // P  # 4 output feature tiles
SUB = 512    # tokens per PSUM group (moving-dim max)
F32 = mybir.dt.float32
BF16 = mybir.dt.bfloat16
NP_BF16 = ml_dtypes.bfloat16

# token-chunk schedule: chunk 0 arrives batched via SWDGE (~11.5us incl the
# software-DGE spin-up) while W streams first on the HWDGE rings (~10.4us,
# gen-rate-bound at ~50 desc/us/ring); later chunks stream on HWDGE; small
# tail chunk for a quick final flush
CHUNKS = [1024, 2048, 2048, 2048, 896, 128]
assert sum(CHUNKS) == TOKENS
CMAX = max(CHUNKS)

_CACHE: dict = {}


def _body(tc, nc, xT, w, outT):
    with (
        tc.tile_pool(name="wp", bufs=1) as wp,
        tc.tile_pool(name="xin", bufs=20) as xin,
        tc.tile_pool(name="outp", bufs=3) as outp,
        tc.tile_pool(name="pso", bufs=8, space="PSUM") as pso,
    ):
        # weights [512, 512] bf16 -> [128, kt, 512]: first in both HWDGE
        # rings (the descriptor-generation head start is what gates group 0)
        w_t = wp.tile([P, KT, N], BF16, tag="wf")
        w_v = w.rearrange("(j p) n -> j p n", p=P)
        for j in range(KT):
            eng = nc.sync if j % 2 == 0 else nc.scalar
            eng.dma_start(w_t[:, j, :], w_v[j])

        # chunk 0 rides SWDGE as a single batched DMA: software descriptor
        # gen is ~60x cheaper per descriptor, landing ~11.5us despite the
        # SWDGE ucode spin-up, while the HWDGE rings are busy with W + x1
        x_pj = xT.rearrange("(j p) m -> p j m", p=P)
        x0_t = wp.tile([P, KT, CHUNKS[0]], BF16, tag="x0")
        nc.gpsimd.dma_start(x0_t[:], x_pj[:, :, 0:CHUNKS[0]])

        # warm-up junk matmuls ramp the PE p-state while chunk-0 data is in
        # flight; the zeroed tile depends only on the early const load
        warm = wp.tile([P, SUB], BF16, tag="warm")
        nc.vector.memset(warm[:], 0.0)
        ps_w = pso.tile([P, SUB], F32, tag="pso")
        for _ in range(12):
            nc.tensor.matmul(ps_w[:], warm[:, :P], warm[:], start=True, stop=True)

        m0 = 0
        for ci, c in enumerate(CHUNKS):
            if ci == 0:
                xs = [x0_t[:, j, :] for j in range(KT)]
            else:
                # later chunks stream on the two HWDGE rings (sync=SP,
                # scalar=ACT), two k-tiles per ring
                xs = []
                for j in range(KT):
                    x_t = xin.tile([P, CMAX], BF16, tag="x")
                    eng = nc.sync if j % 2 == 0 else nc.scalar
                    eng.dma_start(x_t[:, :c], xT[j * P:(j + 1) * P, m0:m0 + c])
                    xs.append(x_t[:, :])

            ots = [outp.tile([P, CMAX], BF16, tag=f"o{nt}", name=f"ot{nt}") for nt in range(NT)]
            for s0 in range(0, c, SUB):
                sw = min(SUB, c - s0)
                for nt in range(NT):
                    ps_o = pso.tile([P, SUB], F32, tag="pso")
                    for j in range(KT):
                        nc.tensor.matmul(
                            ps_o[:, :sw],
                            w_t[:, j, nt * P:(nt + 1) * P],
                            xs[j][:, s0:s0 + sw],
                            start=(j == 0),
                            stop=(j == KT - 1),
                        )
                    # split the PSUM drain into two half-tiles on DVE + ACT so
                    # the bank recycles ~2x sooner (frees the PE 2 groups on)
                    h = sw // 2
                    nc.vector.tensor_copy(ots[nt][:, s0:s0 + h], ps_o[:, :h])
                    nc.scalar.copy(ots[nt][:, s0 + h:s0 + sw], ps_o[:, h:sw])
            # flush the chunk on the SWDGE ring; the last two chunks ride the
            # HWDGE rings (input traffic is done by then) so the SWDGE queue
            # drains while the tail still computes
            for nt in range(NT):
                if ci >= len(CHUNKS) - 2:
                    eng = nc.sync if nt % 2 == 0 else nc.scalar
                else:
                    eng = nc.gpsimd
                eng.dma_start(outT[nt * P:(nt + 1) * P, m0:m0 + c], ots[nt][:, :c])
            m0 += c


def _build():
    nc = bacc.Bacc("TRN2", target_bir_lowering=False, debug=False, num_devices=G)
    xT = nc.dram_tensor("xT", [M, TOKENS], BF16, kind="ExternalInput").ap()
    w = nc.dram_tensor("w", [M, N], BF16, kind="ExternalInput").ap()
    outT = nc.dram_tensor("outT", [N, TOKENS], BF16, kind="ExternalOutput").ap()
    with tile.TileContext(nc) as tc:
        _body(tc, nc, xT, w, outT)
    nc.compile()
    return nc


def _run(in_maps, **kwargs):
    if "nc" not in _CACHE:
        _CACHE["nc"] = _build()
    return run_bass_kernel_spmd(_CACHE["nc"], in_maps, list(range(G)), **kwargs)


def _in_maps(x, blocks):
    return [
        {
            "xT": np.ascontiguousarray(x[:, g * M:(g + 1) * M].T).astype(NP_BF16),
            "w": np.ascontiguousarray(blocks[g]).astype(NP_BF16),
        }
        for g in range(G)
    ]


def kernel(x, blocks):
    x = np.asarray(x, dtype=np.float32)
    blocks = np.asarray(blocks, dtype=np.float32)
    res = _run(_in_maps(x, blocks))
    return np.concatenate(
        [res.results[g]["outT"].T for g in range(G)], axis=1
    ).astype(np.float32)
